# revision 33
# baseline (speedup 1.0000x reference)
"""Neighbor-slice attention (nn_AttentionModule) on 8 TRN2 NeuronCores.

v4 layout strategy (per core, 2 of 16 slices + 1 halo slice each side packed
by the host):
  - host sends x4h fp16 [4, C, hw] for projections (fp16 matmul = 1 cyc/row)
    and x2 f32 [2, C, hw] for the residual add of the local slices.
  - qT/kT projections:   (64 ci, hw) fp16 via matmul lhsT=[WqT|WkT] fp16
  - v projection:        (hw-chunk 128, ci) bf16 with a ones column
                         appended -> y matmul yields softmax denominators
                         for free (row 64)
  - attention (per q-block, software-pipelined by one k-chunk):
        LDW(kt_j)  f_j blocks      (fp16, PSUM f32)
        LDW(vg_{j-1}) y_{j-1} blocks  (bf16 accumulate)
        exp_j on ACT (odd j) or DVE Schraudolph int-trick (even j)
    Explicit ldweights + non-self-loading matmuls: back-to-back matmuls
    with unchanged PE weights avoid the ~270ns weight-swap stall.
  - normalize AFTER Wz:  z evacuated to SBUF early (frees PSUM), then
                         out += z_unnorm * bcast(1/d); 1/d via
                         reciprocal_approx_fast (single custom DVE op,
                         input staged to SBUF partition 0 - custom DVE
                         ops ignore the AP base partition on HW)
  - z matmuls of a q-block are deferred into the next q-block's pipeline
    so the tensor queue never head-of-line blocks on the ysb evac.
  - biases: bq/bk applied on PSUM evac; bv/bz folded into 2*(Wz@bv+bz)
"""

import sys

for _p in ("/opt/trn_rl_repo",):
    if _p not in sys.path:
        sys.path.insert(0, _p)

import numpy as np

N_FULL, C, H, W = 16, 128, 48, 48
HW = H * W            # 2304
CI = C // 2           # 64
KC = HW // 128        # 18 k-chunks per slice
NCORES = 8
NLOC = N_FULL // NCORES  # 2 local slices per core

# q-blocks (start, width); 1024+1024+256 = 2304, no padding
QBS = [(0, 1024), (1024, 1024), (2048, 256)]

# Exp engine split: True -> ACT exp, False -> DVE Schraudolph fast-exp.
EXP_ACT_PRED = lambda att, qi, j: (j % 2 == 1)

# bf16 Schraudolph constants: bits16 = round(x*log2(e)*128 + B16)
_S16 = 184.66496736235803          # 2**7 / ln(2)
_B16 = 16256.0 - 4.75              # 127*2**7 with mid-sawtooth correction

_NC_CACHE = {}
LAST_RESULTS = None
TRACE = False


def _build_nc():
    import concourse.bass as bass
    import concourse.mybir as mybir
    import concourse.tile as tile
    from concourse import bacc

    f32 = mybir.dt.float32
    f16 = mybir.dt.float16
    bf16 = mybir.dt.bfloat16
    i16 = mybir.dt.int16
    FT = mybir.ActivationFunctionType

    nc = bacc.Bacc()

    x2_d = nc.declare_dram_parameter("x2", [NLOC, C, HW], f32, isOutput=False)
    x4h_d = nc.declare_dram_parameter("x4h", [4, C, HW], f16, isOutput=False)
    wqk_d = nc.declare_dram_parameter("wqk", [C, C], f16, isOutput=False)
    wv_d = nc.declare_dram_parameter("wv", [C, CI], f16, isOutput=False)
    wz_d = nc.declare_dram_parameter("wz", [CI, C], bf16, isOutput=False)
    bqk_d = nc.declare_dram_parameter("bqk", [C, 1], f32, isOutput=False)
    c2_d = nc.declare_dram_parameter("c2", [C, 1], f32, isOutput=False)
    out_d = nc.declare_dram_parameter("out", [NLOC, C, HW], f32, isOutput=True)

    def noload(m):
        m.ins.ldweights = False

    with tile.TileContext(nc) as tc:
        with tc.tile_pool(name="const", bufs=1) as cpool, \
             tc.tile_pool(name="xt", bufs=2) as xpool, \
             tc.tile_pool(name="xh", bufs=4) as xhpool, \
             tc.tile_pool(name="qt", bufs=2) as qtpool, \
             tc.tile_pool(name="kt", bufs=4) as ktpool, \
             tc.tile_pool(name="vg", bufs=4) as vgpool, \
             tc.tile_pool(name="at", bufs=3) as atpool, \
             tc.tile_pool(name="ysb", bufs=2) as ypool, \
             tc.tile_pool(name="rb", bufs=6) as rbpool, \
             tc.tile_pool(name="u0", bufs=4) as u0pool, \
             tc.tile_pool(name="u1", bufs=2) as u1pool, \
             tc.tile_pool(name="osb", bufs=4) as opool:

            # ---- constants ----
            wqk_t = cpool.tile([C, C], f16, tag="wqk")
            wv_t = cpool.tile([C, CI], f16, tag="wv")
            wz_t = cpool.tile([CI, C], bf16, tag="wz")
            bqk_t = cpool.tile([C, 1], f32, tag="bqk")
            c2_t = cpool.tile([C, 1], f32, tag="c2")

            nc.sync.dma_start(out=wqk_t, in_=wqk_d[:, :])
            nc.sync.dma_start(out=wv_t, in_=wv_d[:, :])
            nc.sync.dma_start(out=wz_t, in_=wz_d[:, :])
            nc.sync.dma_start(out=bqk_t, in_=bqk_d[:, :])
            nc.sync.dma_start(out=c2_t, in_=c2_d[:, :])

            # ---- load features (split DMAs for queue parallelism) ----
            xh_t = []
            for s in range(4):
                xh = xhpool.tile([C, HW], f16, tag="xh")
                nc.sync.dma_start(out=xh[:, 0:HW // 2], in_=x4h_d[s][:, 0:HW // 2])
                nc.sync.dma_start(out=xh[:, HW // 2:HW], in_=x4h_d[s][:, HW // 2:HW])
                xh_t.append(xh)
            x_t = []
            for n in range(NLOC):
                xt = xpool.tile([C, HW], f32, tag="xt")
                nc.sync.dma_start(out=xt[:, 0:HW // 2], in_=x2_d[n][:, 0:HW // 2])
                nc.sync.dma_start(out=xt[:, HW // 2:HW], in_=x2_d[n][:, HW // 2:HW])
                x_t.append(xt)

            # ---- projections ----
            qt_t = [None, None]      # local slices only (x4 idx 1, 2)
            kt_t = [None] * 4
            vg_t = [None] * 4
            with tc.tile_pool(name="pp", bufs=2, space="PSUM") as pp, \
                 tc.tile_pool(name="pv", bufs=2, space="PSUM") as pv:
                for s in range(4):
                    if s in (1, 2):
                        qt = qtpool.tile([CI, HW], f16, tag="qt")
                        qt_t[s - 1] = qt
                    kt = ktpool.tile([CI, HW], f16, tag="kt")
                    kt_t[s] = kt
                    for b0 in range(0, HW, 512):
                        bw = min(512, HW - b0)
                        pq = pp.tile([C, 512], f32, tag="pp")
                        nc.tensor.matmul(pq[:, 0:bw], lhsT=wqk_t,
                                         rhs=xh_t[s][:, b0:b0 + bw],
                                         start=True, stop=True)
                        if s in (1, 2):
                            nc.scalar.activation(
                                qt_t[s - 1][:, b0:b0 + bw],
                                pq[0:CI, 0:bw], FT.Identity,
                                bias=bqk_t[0:CI, :])
                        nc.vector.tensor_scalar_add(
                            kt[:, b0:b0 + bw], pq[CI:C, 0:bw],
                            bqk_t[CI:C, :])
                    # v proj: weights change per chunk, self-loading
                    pvt = pv.tile([C, KC * CI], f32, tag="pv")
                    for j in range(KC):
                        nc.tensor.matmul(pvt[:, CI * j:CI * (j + 1)],
                                         lhsT=xh_t[s][:, 128 * j:128 * (j + 1)],
                                         rhs=wv_t, start=True, stop=True)
                    vg = vgpool.tile([C, KC, CI + 1], bf16, tag="vg")
                    nc.scalar.activation(
                        vg[:, :, 0:CI],
                        pvt.rearrange("p (j d) -> p j d", d=CI), FT.Copy)
                    nc.gpsimd.memset(vg[:, :, CI], 1.0)
                    vg_t[s] = vg

            # ---- attention ----
            with tc.tile_pool(name="pf", bufs=2, space="PSUM") as pf, \
                 tc.tile_pool(name="pacc", bufs=2, space="PSUM") as pacc:
                pending = []       # deferred z + normalize emitters

                def make_tail(n, side, qi, q0, w, yps, ub):
                    def emit():
                        dcp = rbpool.tile([1, w], f32, tag="dcp")
                        nc.scalar.activation(dcp, yps[CI:CI + 1, :], FT.Copy)
                        rbt = rbpool.tile([1, w], f32, tag="rb")
                        nc.vector.reciprocal_approx_fast(rbt, dcp)
                        ysb = ypool.tile([CI, w], bf16, tag="ysb")
                        nc.scalar.activation(ysb, yps[0:CI, :], FT.Copy)
                        zps = pacc.tile([C, w], f32, tag="acc")
                        for b in range(0, w, 512):
                            e = min(b + 512, w)
                            nc.tensor.matmul(zps[:, b:e], lhsT=wz_t,
                                             rhs=ysb[:, b:e],
                                             start=True, stop=True)
                        zsb = opool.tile([C, w], f32, tag="osb")
                        nc.scalar.activation(zsb, zps, FT.Copy)
                        rbc = rbpool.tile([C, w], f32, tag="rbc")
                        nc.gpsimd.partition_broadcast(rbc, rbt)
                        if side == 0:
                            u = u1pool.tile([C, w], f32, tag="u1")
                            nc.vector.tensor_mul(u, zsb, rbc)
                            t0 = u0pool.tile([C, w], f32, tag="u0")
                            nc.vector.scalar_tensor_tensor(
                                out=t0, in0=u, scalar=c2_t,
                                in1=x_t[n][:, q0:q0 + w],
                                op0=mybir.AluOpType.add,
                                op1=mybir.AluOpType.add)
                            ub[qi] = t0
                        else:
                            ua = u1pool.tile([C, w], f32, tag="u1")
                            nc.vector.tensor_mul(ua, zsb, rbc)
                            osb = opool.tile([C, w], f32, tag="osb")
                            nc.vector.tensor_add(osb, ua, ub[qi])
                            nc.sync.dma_start(out=out_d[n][:, q0:q0 + w],
                                              in_=osb)
                    return emit

                att = 0
                for n in range(NLOC):
                    ub = [None, None, None]
                    for side in range(2):          # 0: before, 1: after
                        kv = n + (0 if side == 0 else 2)
                        for qi, (q0, w) in enumerate(QBS):
                            yps = pacc.tile([CI + 1, w], f32, tag="acc")
                            at_prev = None
                            for j in range(KC):
                                ktc = kt_t[kv][:, 128 * j:128 * (j + 1)]
                                ft = pf.tile([C, w], f32, tag="ft")
                                for b in range(0, w, 512):
                                    e = min(b + 512, w)
                                    nc.tensor.matmul(
                                        ft[:, b:e], lhsT=ktc,
                                        rhs=qt_t[n][:, q0 + b:q0 + e],
                                        start=True, stop=True)
                                if at_prev is not None:
                                    vgc = vg_t[kv][:, j - 1, :]
                                    for b in range(0, w, 512):
                                        e = min(b + 512, w)
                                        nc.tensor.matmul(
                                            yps[:, b:e], lhsT=vgc,
                                            rhs=at_prev[:, b:e],
                                            start=(j == 1), stop=False,
                                            skip_group_check=True)
                                if j == 1 and pending:
                                    pending.pop(0)()
                                at = atpool.tile([C, w], bf16, tag="at")
                                if EXP_ACT_PRED(att, qi, j):
                                    nc.scalar.activation(at, ft, FT.Exp)
                                else:
                                    nc.vector.tensor_scalar(
                                        at.bitcast(i16), ft, _S16, _B16,
                                        op0=mybir.AluOpType.mult,
                                        op1=mybir.AluOpType.add)
                                at_prev = at
                            vgc = vg_t[kv][:, KC - 1, :]
                            for b in range(0, w, 512):
                                e = min(b + 512, w)
                                nc.tensor.matmul(
                                    yps[:, b:e], lhsT=vgc,
                                    rhs=at_prev[:, b:e],
                                    start=False, stop=True,
                                    skip_group_check=True)
                            pending.append(
                                make_tail(n, side, qi, q0, w, yps, ub))
                        att += 1
                while pending:
                    pending.pop(0)()

    nc.compile()
    return nc


def _get_nc():
    if "nc" not in _NC_CACHE:
        _NC_CACHE["nc"] = _build_nc()
    return _NC_CACHE["nc"]


def _host_inputs(features, Wq, bq, Wk, bk, Wv, bv, Wz, bz):
    import ml_dtypes

    X = np.ascontiguousarray(np.asarray(features, np.float32).reshape(N_FULL, C, HW))
    Xh = X.astype(np.float16)
    wqk = np.ascontiguousarray(
        np.concatenate([Wq.T, Wk.T], axis=1)).astype(np.float16)
    wv = np.ascontiguousarray(np.asarray(Wv).T).astype(np.float16)
    wz = np.ascontiguousarray(np.asarray(Wz).T).astype(ml_dtypes.bfloat16)
    bqk = np.concatenate([bq, bk]).astype(np.float32).reshape(C, 1)
    c2 = (2.0 * (np.asarray(Wz) @ np.asarray(bv) + np.asarray(bz))).astype(
        np.float32).reshape(C, 1)
    in_maps = []
    for i in range(NCORES):
        idx = [max(2 * i - 1, 0), 2 * i, 2 * i + 1, min(2 * i + 2, N_FULL - 1)]
        in_maps.append({
            "x2": np.ascontiguousarray(X[2 * i:2 * i + 2]),
            "x4h": np.ascontiguousarray(Xh[idx]),
            "wqk": wqk, "wv": wv, "wz": wz, "bqk": bqk, "c2": c2,
        })
    return in_maps


def kernel(features, Wq, bq, Wk, bk, Wv, bv, Wz, bz):
    global LAST_RESULTS
    from concourse.bass_utils import run_bass_kernel_spmd

    nc = _get_nc()
    in_maps = _host_inputs(features, Wq, bq, Wk, bk, Wv, bv, Wz, bz)
    res = run_bass_kernel_spmd(nc, in_maps, core_ids=list(range(NCORES)),
                               trace=TRACE)
    LAST_RESULTS = res
    out = np.empty((N_FULL, C, H, W), np.float32)
    for i in range(NCORES):
        out[2 * i:2 * i + 2] = res.results[i]["out"].reshape(NLOC, C, H, W)
    return out
